# revision 1
# baseline (speedup 1.0000x reference)
"""BitNet attention layer on 8 Trainium2 NeuronCores.

Tensor-parallel over heads: core i owns heads {2i, 2i+1}. Each core:
  - computes q^T,k^T (feature-major) + v (natural) for its heads via fp32r
    matmuls against host-pretransposed x^T and ternary-quantized W^T slices
  - RoPE on q^T/k^T (partition-dim rotate-half, sign folded into sin table)
  - causal attention with transposed scores S^T[k,q] (softmax denominator via
    ones-matmul partition reduce; diagonal blocks masked multiplicatively)
  - o_proj partial over its 256 ctx features -> fp16 partial [2048, 2048]
Host sums the 8 partials.

All matmuls run in float32r (tf32-class, 1 cycle/row at free dim >= 256).
Tiles are split per seq-tile / h-chunk-group so the Tile scheduler can
overlap projection, attention, and o_proj phases.
"""
import os
import sys

import numpy as np

try:
    import concourse.bass as bass
except ImportError:
    sys.path.insert(0, "/opt/trn_rl_repo")
    import concourse.bass as bass

import concourse.mybir as mybir
import concourse.tile as tile
from concourse import bacc
from concourse.bass_utils import run_bass_kernel_spmd

F32 = mybir.dt.float32
F32R = mybir.dt.float32r
F16 = mybir.dt.float16
BF16 = mybir.dt.bfloat16

S = 2048          # sequence length
H = 2048          # hidden
D = 128           # head dim
NCORES = 8
HPC = 2           # heads per core
OC = 3 * HPC * D  # 768 per-core projection output features (q|k|v)
ST = 512          # seq tile for projection rhs / attention qi tile
NST = S // ST     # 4
HC = H // 128     # 16 h-chunks
HG = 4            # h-chunk group size (DMA granularity)
NG = HC // HG     # 4 groups
NKJ = S // 128    # 16 kj chunks
ROPE_BASE = 10000.0

_built = None
_PHASES = os.environ.get("KPH", "ABC")


def _build(timing=False):
    nc = bacc.Bacc("TRN2", target_bir_lowering=False, debug=False,
                   dynamic_dma_scratch_size=4096)

    if timing:
        # timing variant: identical device work, but big tensors live in
        # internal DRAM (garbage data) so per-call host<->device transfer is
        # tiny and wall-clock deltas measure the NEFF itself.
        xt_d = nc.dram_tensor("xt_i", [H, S], F32R)
        wt_d = nc.dram_tensor("wt_i", [H, OC], F32R)
        wot_d = nc.dram_tensor("wot_i", [HPC * D, H], F32R)
        cos_d = nc.dram_tensor("cost_i", [D, S], F32)
        sin_d = nc.dram_tensor("sins_i", [D, S], F32)
        tri_d = nc.dram_tensor("tri_i", [128, 896], BF16)
        out_d = nc.dram_tensor("out_i", [S, H], F16)
        out_x = nc.declare_dram_parameter("out", [128, H], F16, isOutput=True)
    else:
        xt_d = nc.declare_dram_parameter("xt", [H, S], F32R, isOutput=False)
        wt_d = nc.declare_dram_parameter("wt", [H, OC], F32R, isOutput=False)
        wot_d = nc.declare_dram_parameter("wot", [HPC * D, H], F32R,
                                          isOutput=False)
        cos_d = nc.declare_dram_parameter("cost", [D, S], F32, isOutput=False)
        sin_d = nc.declare_dram_parameter("sins", [D, S], F32, isOutput=False)
        tri_d = nc.declare_dram_parameter("tri", [128, 896], BF16,
                                          isOutput=False)
        out_d = nc.declare_dram_parameter("out", [S, H], F16, isOutput=True)
    onc_d = nc.declare_dram_parameter("onc", [128, 1], F32R, isOutput=False)
    onr_d = nc.declare_dram_parameter("onr", [1, 128], F32R, isOutput=False)
    osq_d = nc.declare_dram_parameter("osq", [128, 128], F32R, isOutput=False)

    # exp scale (s_p^2/sqrt(D)) and output scale (s_p*s_o) are runtime values;
    # pass them as tiny per-partition inputs instead of baking into the NEFF.
    esc_d = nc.declare_dram_parameter("esc", [128, 1], F32, isOutput=False)
    osc_d = nc.declare_dram_parameter("osc", [128, 1], F32, isOutput=False)

    with tile.TileContext(nc) as tc, nc.allow_low_precision(
        reason="float32r rounding for PE matmul operands"
    ):
        with tc.tile_pool(name="const", bufs=1) as cpool, \
             tc.tile_pool(name="qkv", bufs=1) as qpool, \
             tc.tile_pool(name="ctx", bufs=1) as xpool, \
             tc.tile_pool(name="wo", bufs=1) as wopool, \
             tc.tile_pool(name="ob", bufs=2) as opool:
            cost = cpool.tile([D, S], F32)
            sins = cpool.tile([D, S], F32)
            tri = cpool.tile([128, 896], BF16)
            onc = cpool.tile([128, 1], F32R)
            onr = cpool.tile([1, 128], F32R)
            osq = cpool.tile([128, 128], F32R)
            esc = cpool.tile([128, 1], F32)
            osc = cpool.tile([128, 1], F32)
            nc.sync.dma_start(onc[:], onc_d[:])
            nc.sync.dma_start(onr[:], onr_d[:])
            nc.sync.dma_start(osq[:], osq_d[:])
            nc.sync.dma_start(esc[:], esc_d[:])
            nc.sync.dma_start(osc[:], osc_d[:])
            wot = wopool.tile([128, HPC, H], F32R)

            # persistent per-head tensors, tiled per seq-tile for fine deps
            qk = [[qpool.tile([D, ST], F32R, name=f"qk{oc}_{st}")
                   for st in range(NST)] for oc in range(4)]
            v_sb = [qpool.tile([128, ST // 128, HPC * D], F32R, name=f"v{st}")
                    for st in range(NST)]
            ctx = [[xpool.tile([D, ST], F32R, name=f"ctx{h}_{t}")
                    for t in range(NST)] for h in range(HPC)]

            # ---------------- Phase A: qkv projection + RoPE ----------------
            if "A" in _PHASES:
             with tc.tile_pool(name="wt", bufs=1) as wpool, \
                 tc.tile_pool(name="xt", bufs=2) as xtpool, \
                 tc.tile_pool(name="ropet", bufs=2) as rpool, \
                 tc.tile_pool(name="psA", bufs=3, space="PSUM") as psA, \
                 tc.tile_pool(name="psV", bufs=2, space="PSUM") as psV:
                wt = [wpool.tile([128, HG, OC], F32R, name=f"wt{g}")
                      for g in range(NG)]
                for g in range(NG):
                    nc.sync.dma_start(
                        wt[g][:],
                        wt_d[g * HG * 128:(g + 1) * HG * 128].rearrange(
                            "(ho hp) o -> hp ho o", hp=128))

                _deferred = [False]

                for st in range(NST):
                    ssl = slice(st * ST, (st + 1) * ST)
                    xt = [xtpool.tile([128, HG, ST], F32R, name=f"xt{g}")
                          for g in range(NG)]
                    for g in range(NG):
                        nc.sync.dma_start(
                            xt[g][:],
                            xt_d[g * HG * 128:(g + 1) * HG * 128, ssl].rearrange(
                                "(ho hp) s -> hp ho s", hp=128))
                    if not _deferred[0]:
                        _deferred[0] = True
                        nc.sync.dma_start(cost[:], cos_d[:])
                        nc.sync.dma_start(sins[:], sin_d[:])
                        nc.sync.dma_start(tri[:], tri_d[:])
                        nc.sync.dma_start(
                            wot[:],
                            wot_d.rearrange("(co cp) o -> cp co o", cp=128))

                    # q,k chunks (features oc*128..): RoPE'd into qk[oc][st]
                    for oc in range(4):
                        ps = psA.tile([128, ST], F32)
                        for hcc in range(HC):
                            nc.tensor.matmul(
                                ps[:],
                                wt[hcc // HG][:, hcc % HG,
                                              oc * 128:(oc + 1) * 128],
                                xt[hcc // HG][:, hcc % HG, :],
                                start=(hcc == 0), stop=(hcc == HC - 1))
                        dst = qk[oc][st]
                        t2 = rpool.tile([128, ST], F32)
                        nc.vector.tensor_mul(t2[0:64, :], ps[64:128, :],
                                             sins[0:64, ssl])
                        nc.vector.tensor_mul(t2[64:128, :], ps[0:64, :],
                                             sins[64:128, ssl])
                        nc.vector.tensor_mul(dst[:], ps[:], cost[:, ssl])
                        nc.vector.tensor_add(dst[:], dst[:], t2[:])

                    # v natural: [s-chunk 128, 256]
                    for sc in range(ST // 128):
                        ps = psV.tile([128, HPC * D], F32)
                        for hcc in range(HC):
                            nc.tensor.matmul(
                                ps[:],
                                xt[hcc // HG][:, hcc % HG,
                                              sc * 128:(sc + 1) * 128],
                                wt[hcc // HG][:, hcc % HG, 4 * 128:],
                                start=(hcc == 0), stop=(hcc == HC - 1))
                        nc.scalar.copy(v_sb[st][:, sc, :], ps[:])

            # ---------- Phase B+C: attention + o_proj, interleaved ----------
            if "B" in _PHASES:
             with tc.tile_pool(name="pt", bufs=2) as ptpool, \
                 tc.tile_pool(name="rden", bufs=2) as dpool, \
                 tc.tile_pool(name="psS", bufs=3, space="PSUM") as psS, \
                 tc.tile_pool(name="psC", bufs=2, space="PSUM") as psC, \
                 tc.tile_pool(name="psB", bufs=1, space="PSUM") as psB, \
                 tc.tile_pool(name="psO", bufs=2, space="PSUM") as psO:
                for t in range(NST):
                    for h in range(HPC):
                        nkj = 4 * (t + 1)
                        pt = ptpool.tile([128, NKJ, ST], F32R)
                        for j in range(nkj):
                            sp = psS.tile([128, ST], F32)
                            nc.tensor.matmul(
                                sp[:],
                                qk[2 + h][j // 4][:, (j % 4) * 128:
                                                  (j % 4 + 1) * 128],
                                qk[h][t][:],
                                start=True, stop=True)
                            # probs (unnormalized): exp(esc * scores)
                            nc.scalar.activation(
                                pt[:, j, :], sp[:],
                                mybir.ActivationFunctionType.Exp,
                                bias=0.0, scale=esc[:])
                            off = 128 * j - ST * t
                            if off >= 0:  # diagonal block: tril mask
                                nc.vector.tensor_mul(
                                    pt[:, j, :], pt[:, j, :],
                                    tri[:, 384 - off:896 - off])
                        # ctx^T[d, qi] accumulate over kj
                        cp = psC.tile([128, ST], F32)
                        for j in range(nkj):
                            nc.tensor.matmul(
                                cp[:], v_sb[j // 4][:, j % 4, h * D:(h + 1) * D],
                                pt[:, j, :],
                                start=(j == 0), stop=(j == nkj - 1))
                        # denominators, broadcast across partitions in one
                        # pass: all-ones [128,128] lhsT -> every out partition
                        # holds sum over kj
                        bp = psB.tile([128, ST], F32)
                        for j in range(nkj):
                            nc.tensor.matmul(bp[:], osq[:], pt[:, j, :],
                                             start=(j == 0), stop=(j == nkj - 1))
                        rbp = dpool.tile([128, ST], F32, name="rbp")
                        nc.vector.reciprocal(rbp[:], bp[:])
                        nc.scalar.copy(ctx[h][t][:], cp[:])
                        nc.vector.tensor_mul(ctx[h][t][:], ctx[h][t][:], rbp[:])

                    # o_proj rows for this t (ctx[*][t] complete)
                    if "C" in _PHASES:
                        for sc in range(4 * t, 4 * t + 4):
                            for half in range(2):
                                ob = opool.tile([128, H // 2], F16)
                                for oth in range(2):
                                    ot = half * 2 + oth
                                    po = psO.tile([128, ST], F32)
                                    for cc in range(HPC):
                                        nc.tensor.matmul(
                                            po[:],
                                            ctx[cc][t][:, (sc % 4) * 128:
                                                       (sc % 4 + 1) * 128],
                                            wot[:, cc, ot * ST:(ot + 1) * ST],
                                            start=(cc == 0),
                                            stop=(cc == HPC - 1))
                                    if ot % 2 == 0:
                                        nc.scalar.activation(
                                            ob[:, oth * ST:(oth + 1) * ST],
                                            po[:],
                                            mybir.ActivationFunctionType.Copy,
                                            bias=0.0, scale=osc[:])
                                    else:
                                        nc.vector.tensor_scalar_mul(
                                            ob[:, oth * ST:(oth + 1) * ST],
                                            po[:], osc[:])
                                nc.sync.dma_start(
                                    out_d[sc * 128:(sc + 1) * 128,
                                          half * (H // 2):(half + 1) * (H // 2)],
                                    ob[:])

            if timing:
                nc.sync.dma_start(out_x[:], out_d[S - 128:, :])

    nc.compile()
    return nc


def _host_prep(hidden_states, w_proj, w_o):
    x = np.asarray(hidden_states, dtype=np.float32).reshape(S, H)
    w_proj = np.asarray(w_proj, dtype=np.float32)
    w_o = np.asarray(w_o, dtype=np.float32)

    # BitNet b1.58 per-tensor absmean quantization (ternary, scale factored out)
    s_p = np.float32(np.mean(np.abs(w_proj), dtype=np.float32)) + np.float32(1e-5)
    s_o = np.float32(np.mean(np.abs(w_o), dtype=np.float32)) + np.float32(1e-5)
    tp = np.clip(np.round(w_proj / s_p), -1.0, 1.0).astype(np.float32)
    to = np.clip(np.round(w_o / s_o), -1.0, 1.0).astype(np.float32)

    xt = np.ascontiguousarray(x.T)                      # [H, S]

    # RoPE tables, feature-major, rotate-half sign folded into sin
    inv_freq = (1.0 / (ROPE_BASE ** (np.arange(0, D, 2, dtype=np.float32) / D))
                ).astype(np.float32)
    t = np.arange(S, dtype=np.float32)
    freqs = np.outer(inv_freq, t).astype(np.float32)    # [64, S]
    cosT = np.concatenate([np.cos(freqs), np.cos(freqs)], 0).astype(np.float32)
    sinS = np.concatenate([-np.sin(freqs), np.sin(freqs)], 0).astype(np.float32)

    # shifted tril mask bank: tri[p, x] = 1 if p <= x - 384
    p = np.arange(128)[:, None]
    xx = np.arange(896)[None, :]
    import ml_dtypes
    tri = (p <= xx - 384).astype(ml_dtypes.bfloat16)

    esc = np.full((128, 1), s_p * s_p / np.sqrt(np.float32(D)), np.float32)
    osc = np.full((128, 1), s_p * s_o, np.float32)
    onc = np.ones((128, 1), np.float32)
    onr = np.ones((1, 128), np.float32)

    in_maps = []
    for c in range(NCORES):
        r = slice(c * HPC * D, (c + 1) * HPC * D)       # 256 features
        wt_c = np.ascontiguousarray(
            np.concatenate([tp[:H][r], tp[H:2 * H][r], tp[2 * H:][r]], 0).T)
        wot_c = np.ascontiguousarray(to[:, r].T)        # [256, H]
        in_maps.append({
            "xt": xt, "wt": wt_c, "wot": wot_c, "cost": cosT, "sins": sinS,
            "tri": tri, "onc": onc, "onr": onr, "osq": np.ones((128, 128), np.float32), "esc": esc, "osc": osc,
        })
    return in_maps


def kernel(hidden_states, attention_mask, w_proj, w_o):
    global _built
    if _built is None:
        _built = _build()
    nc = _built
    in_maps = _host_prep(hidden_states, w_proj, w_o)
    res = run_bass_kernel_spmd(nc, in_maps, core_ids=list(range(NCORES)))
    acc = np.zeros((S, H), np.float32)
    for c in range(NCORES):
        acc += res.results[c]["out"].astype(np.float32)
    return acc.reshape(1, S, H)



# revision 8
# speedup vs baseline: 1.2186x; 1.2186x over previous
"""BitNet attention layer on 8 Trainium2 NeuronCores.

Tensor-parallel over heads: core i owns heads {2i, 2i+1}. Each core:
  - computes q^T,k^T (feature-major) + v (natural) for its heads via bf16
    matmuls against host-pretransposed x^T and ternary-quantized W^T slices
  - RoPE on q^T/k^T (partition-dim rotate-half, sign folded into sin table)
  - causal attention with transposed scores S^T[k,q]; diagonal 128-blocks use
    triangular free-dim slices so fully-masked columns are never computed;
    softmax denominator via elementwise chunk pre-sum on DVE + a single
    ones-matmul partition reduce per (tile, head)
  - o_proj partial over its 256 ctx features -> fp16 partial [2048, 2048]
Host sums the 8 partials.

All matmuls run with bf16 operands (1 cycle/row on the PE at any free size).
DMA: weight and x^T tiles stream interleaved so the first projection chain
starts ~5us in; the first seq-tile consumes h-chunk groups as they arrive.
"""
import os
import sys

import numpy as np

try:
    import concourse.bass as bass
except ImportError:
    sys.path.insert(0, "/opt/trn_rl_repo")
    import concourse.bass as bass

import concourse.mybir as mybir
import concourse.tile as tile
from concourse import bacc
from concourse.bass_utils import run_bass_kernel_spmd

F32 = mybir.dt.float32
F32R = mybir.dt.float32r
F16 = mybir.dt.float16
BF16 = mybir.dt.bfloat16

S = 2048          # sequence length
H = 2048          # hidden
D = 128           # head dim
NCORES = 8
HPC = 2           # heads per core
OC = 3 * HPC * D  # 768 per-core projection output features (q|k|v)
ST = 512          # seq tile for projection rhs / attention qi tile
NST = S // ST     # 4
HC = H // 128     # 16 h-chunks
HG = 4            # h-chunk group size (DMA granularity)
NG = HC // HG     # 4 groups
ROPE_BASE = 10000.0

_built = None
_PHASES = os.environ.get("KPH", "ABC")


def _build(timing=False):
    nc = bacc.Bacc("TRN2", target_bir_lowering=False, debug=False,
                   dynamic_dma_scratch_size=4096)

    if timing:
        # timing variant: identical device work, but big tensors live in
        # internal DRAM (garbage data) so per-call host<->device transfer is
        # tiny and wall-clock deltas measure the NEFF itself.
        xt_d = nc.dram_tensor("xt_i", [H, S], BF16)
        wt_d = nc.dram_tensor("wt_i", [H, OC], BF16)
        wot_d = nc.dram_tensor("wot_i", [HPC * D, H], BF16)
        cos_d = nc.dram_tensor("cost_i", [D, S], F32)
        sin_d = nc.dram_tensor("sins_i", [D, S], F32)
        tri_d = nc.dram_tensor("tri_i", [128, 896], BF16)
        out_d = nc.dram_tensor("out_i", [S, H], F16)
        out_x = nc.declare_dram_parameter("out", [128, H], F16, isOutput=True)
    else:
        xt_d = nc.declare_dram_parameter("xt", [H, S], BF16, isOutput=False)
        wt_d = nc.declare_dram_parameter("wt", [H, OC], BF16, isOutput=False)
        wot_d = nc.declare_dram_parameter("wot", [HPC * D, H], BF16,
                                          isOutput=False)
        cos_d = nc.declare_dram_parameter("cost", [D, S], F32, isOutput=False)
        sin_d = nc.declare_dram_parameter("sins", [D, S], F32, isOutput=False)
        tri_d = nc.declare_dram_parameter("tri", [128, 896], BF16,
                                          isOutput=False)
        out_d = nc.declare_dram_parameter("out", [S, H], F16, isOutput=True)
    osq_d = nc.declare_dram_parameter("osq", [128, 128], BF16, isOutput=False)

    # exp scale (s_p^2/sqrt(D)) and output scale (s_p*s_o) are runtime values;
    # pass them as tiny per-partition inputs instead of baking into the NEFF.
    esc_d = nc.declare_dram_parameter("esc", [128, 1], F32, isOutput=False)
    osc_d = nc.declare_dram_parameter("osc", [128, 1], F32, isOutput=False)

    with tile.TileContext(nc) as tc, nc.allow_low_precision(
        reason="bf16 matmul operands / probs; validated 3.3e-3 rel err"
    ):
        with tc.tile_pool(name="const", bufs=1) as cpool, \
             tc.tile_pool(name="qkv", bufs=1) as qpool, \
             tc.tile_pool(name="ctx", bufs=1) as xpool, \
             tc.tile_pool(name="wo", bufs=1) as wopool, \
             tc.tile_pool(name="ob", bufs=2) as opool:
            cost = cpool.tile([D, S], F32)
            sins = cpool.tile([D, S], F32)
            tri = cpool.tile([128, 896], BF16)
            osq = cpool.tile([128, 128], BF16)
            esc = cpool.tile([128, 1], F32)
            osc = cpool.tile([128, 1], F32)
            wot = wopool.tile([128, HPC, H], BF16)

            # persistent per-head tensors, tiled per seq-tile for fine deps
            qk = [[qpool.tile([D, ST], BF16, name=f"qk{oc}_{st}")
                   for st in range(NST)] for oc in range(4)]
            v_sb = [qpool.tile([128, ST // 128, HPC * D], BF16, name=f"v{st}")
                    for st in range(NST)]
            ctx = [[xpool.tile([D, ST], BF16, name=f"ctx{h}_{t}")
                    for t in range(NST)] for h in range(HPC)]

            # ---------------- Phase A: qkv projection + RoPE ----------------
            if "A" in _PHASES:
             with tc.tile_pool(name="wt", bufs=1) as wpool, \
                 tc.tile_pool(name="xt", bufs=2) as xtpool, \
                 tc.tile_pool(name="ropet", bufs=2) as rpool, \
                 tc.tile_pool(name="psA", bufs=4, space="PSUM") as psA, \
                 tc.tile_pool(name="psV", bufs=2, space="PSUM") as psV:
                wt = [wpool.tile([128, HG, OC], BF16, name=f"wt{g}")
                      for g in range(NG)]
                xt0 = [xtpool.tile([128, HG, ST], BF16, name=f"xt{g}")
                       for g in range(NG)]
                # critical startup stream: weight group then matching x group
                for g in range(NG):
                    nc.sync.dma_start(
                        wt[g][:],
                        wt_d[g * HG * 128:(g + 1) * HG * 128].rearrange(
                            "(ho hp) o -> hp ho o", hp=128))
                    nc.sync.dma_start(
                        xt0[g][:],
                        xt_d[g * HG * 128:(g + 1) * HG * 128, 0:ST].rearrange(
                            "(ho hp) s -> hp ho s", hp=128))
                # non-critical constants after the startup stream
                nc.sync.dma_start(cost[:], cos_d[:])
                nc.sync.dma_start(sins[:], sin_d[:])
                nc.sync.dma_start(tri[:], tri_d[:])
                nc.sync.dma_start(
                    wot[:],
                    wot_d.rearrange("(co cp) o -> cp co o", cp=128))
                nc.sync.dma_start(esc[:], esc_d[:])
                nc.sync.dma_start(osc[:], osc_d[:])
                nc.sync.dma_start(osq[:], osq_d[:])

                def rope(dst, ps, ssl):
                    t2 = rpool.tile([128, ST], BF16, name="t2")
                    nc.vector.tensor_mul(t2[0:64, :], ps[64:128, :],
                                         sins[0:64, ssl])
                    nc.vector.tensor_mul(t2[64:128, :], ps[0:64, :],
                                         sins[64:128, ssl])
                    nc.vector.tensor_mul(dst[:], ps[:], cost[:, ssl])
                    nc.vector.tensor_add(dst[:], dst[:], t2[:])

                def v_pass(st, xt):
                    for sc in range(ST // 128):
                        ps = psV.tile([128, HPC * D], F32)
                        for hcc in range(HC):
                            nc.tensor.matmul(
                                ps[:],
                                xt[hcc // HG][:, hcc % HG,
                                              sc * 128:(sc + 1) * 128],
                                wt[hcc // HG][:, hcc % HG, 4 * 128:],
                                start=(hcc == 0), stop=(hcc == HC - 1))
                        nc.scalar.copy(v_sb[st][:, sc, :], ps[:])

                # st0: h-chunk-major so the PE consumes groups as they arrive
                psA0 = [psA.tile([128, ST], F32, name="ps") for oc in range(4)]
                for g in range(NG):
                    for hq in range(HG):
                        hcc = g * HG + hq
                        for oc in range(4):
                            nc.tensor.matmul(
                                psA0[oc][:],
                                wt[g][:, hq, oc * 128:(oc + 1) * 128],
                                xt0[g][:, hq, :],
                                start=(hcc == 0), stop=(hcc == HC - 1))
                for oc in range(4):
                    rope(qk[oc][0], psA0[oc], slice(0, ST))
                v_pass(0, xt0)

                # st1..3: oc-major chains, xt double-buffered
                for st in range(1, NST):
                    ssl = slice(st * ST, (st + 1) * ST)
                    xt = [xtpool.tile([128, HG, ST], BF16, name=f"xt{g}")
                          for g in range(NG)]
                    for g in range(NG):
                        nc.sync.dma_start(
                            xt[g][:],
                            xt_d[g * HG * 128:(g + 1) * HG * 128, ssl].rearrange(
                                "(ho hp) s -> hp ho s", hp=128))
                    for oc in range(4):
                        ps = psA.tile([128, ST], F32, name="ps")
                        for hcc in range(HC):
                            nc.tensor.matmul(
                                ps[:],
                                wt[hcc // HG][:, hcc % HG,
                                              oc * 128:(oc + 1) * 128],
                                xt[hcc // HG][:, hcc % HG, :],
                                start=(hcc == 0), stop=(hcc == HC - 1))
                        rope(qk[oc][st], ps, ssl)
                    v_pass(st, xt)

            # ---------- Phase B+C: attention + o_proj, interleaved ----------
            if "B" in _PHASES:
             with tc.tile_pool(name="pt", bufs=1) as ptpool, \
                 tc.tile_pool(name="rden", bufs=2) as dpool, \
                 tc.tile_pool(name="ptsum", bufs=2) as spool, \
                 tc.tile_pool(name="psS", bufs=3, space="PSUM") as psS, \
                 tc.tile_pool(name="psC", bufs=2, space="PSUM") as psC, \
                 tc.tile_pool(name="psB", bufs=1, space="PSUM") as psB, \
                 tc.tile_pool(name="psO", bufs=2, space="PSUM") as psO:
                for t in range(NST):
                    nkj = 4 * (t + 1)
                    for h in range(HPC):
                        pt = ptpool.tile([128, nkj, ST], BF16, name=f"pt{h}")
                        ptsum = spool.tile([128, ST], BF16, name="ptsum")
                        # scores S^T[kj, qi] per 128-chunk; diagonal chunks
                        # only compute the un-masked qi suffix [128i:]
                        for j in range(nkj):
                            di = j - 4 * t       # >= 0 on diagonal chunks
                            lo = 128 * di if di >= 0 else 0
                            sp = psS.tile([128, ST], F32)
                            nc.tensor.matmul(
                                sp[:, lo:],
                                qk[2 + h][j // 4][:, (j % 4) * 128:
                                                  (j % 4 + 1) * 128],
                                qk[h][t][:, lo:],
                                start=True, stop=True)
                            # probs (unnormalized): exp(esc * scores)
                            nc.scalar.activation(
                                pt[:, j, lo:], sp[:, lo:],
                                mybir.ActivationFunctionType.Exp,
                                bias=0.0, scale=esc[:])
                            if di >= 0:  # diagonal block: tril mask
                                nc.vector.tensor_mul(
                                    pt[:, j, lo:], pt[:, j, lo:],
                                    tri[:, 384:896 - lo])
                            # elementwise chunk pre-sum for the softmax
                            # denominator (partition reduce happens once via
                            # the ones-matmul below)
                            if j == 0:
                                nc.vector.tensor_copy(ptsum[:], pt[:, 0, :])
                            else:
                                nc.vector.tensor_add(
                                    ptsum[:, lo:], ptsum[:, lo:],
                                    pt[:, j, lo:])
                        # ctx^T[d, qi] accumulate over kj
                        cp = psC.tile([128, ST], F32)
                        for j in range(nkj):
                            di = j - 4 * t
                            lo = 128 * di if di >= 0 else 0
                            nc.tensor.matmul(
                                cp[:, lo:],
                                v_sb[j // 4][:, j % 4, h * D:(h + 1) * D],
                                pt[:, j, lo:],
                                start=(j == 0), stop=(j == nkj - 1))
                        # denominator: single ones-matmul partition reduce,
                        # broadcast to all 128 out partitions
                        bp = psB.tile([128, ST], F32)
                        nc.tensor.matmul(bp[:], osq[:], ptsum[:],
                                         start=True, stop=True)
                        rbp = dpool.tile([128, ST], F32, name="rbp")
                        nc.vector.reciprocal(rbp[:], bp[:])
                        nc.vector.tensor_mul(ctx[h][t][:], cp[:], rbp[:])

                    # o_proj rows for this t (ctx[*][t] complete); head-0
                    # matmul first in each chain so the PE can proceed while
                    # head-1's ctx normalize drains
                    if "C" in _PHASES:
                        for sc in range(4 * t, 4 * t + 4):
                            for half in range(2):
                                ob = opool.tile([128, H // 2], F16)
                                for oth in range(2):
                                    ot = half * 2 + oth
                                    po = psO.tile([128, ST], F32)
                                    for cc in range(HPC):
                                        nc.tensor.matmul(
                                            po[:],
                                            ctx[cc][t][:, (sc % 4) * 128:
                                                       (sc % 4 + 1) * 128],
                                            wot[:, cc, ot * ST:(ot + 1) * ST],
                                            start=(cc == 0),
                                            stop=(cc == HPC - 1))
                                    if ot % 2 == 0:
                                        nc.scalar.activation(
                                            ob[:, oth * ST:(oth + 1) * ST],
                                            po[:],
                                            mybir.ActivationFunctionType.Copy,
                                            bias=0.0, scale=osc[:])
                                    else:
                                        nc.vector.tensor_scalar_mul(
                                            ob[:, oth * ST:(oth + 1) * ST],
                                            po[:], osc[:])
                                nc.sync.dma_start(
                                    out_d[sc * 128:(sc + 1) * 128,
                                          half * (H // 2):(half + 1) * (H // 2)],
                                    ob[:])

            if timing:
                nc.sync.dma_start(out_x[:], out_d[S - 128:, :])

    nc.compile()
    return nc


def _host_prep(hidden_states, w_proj, w_o):
    import ml_dtypes
    x = np.asarray(hidden_states, dtype=np.float32).reshape(S, H)
    w_proj = np.asarray(w_proj, dtype=np.float32)
    w_o = np.asarray(w_o, dtype=np.float32)

    # BitNet b1.58 per-tensor absmean quantization (ternary, scale factored out)
    s_p = np.float32(np.mean(np.abs(w_proj), dtype=np.float32)) + np.float32(1e-5)
    s_o = np.float32(np.mean(np.abs(w_o), dtype=np.float32)) + np.float32(1e-5)
    tp = np.clip(np.round(w_proj / s_p), -1.0, 1.0).astype(np.float32)
    to = np.clip(np.round(w_o / s_o), -1.0, 1.0).astype(np.float32)

    xt = np.ascontiguousarray(x.T).astype(ml_dtypes.bfloat16)   # [H, S]

    # RoPE tables, feature-major, rotate-half sign folded into sin
    inv_freq = (1.0 / (ROPE_BASE ** (np.arange(0, D, 2, dtype=np.float32) / D))
                ).astype(np.float32)
    t = np.arange(S, dtype=np.float32)
    freqs = np.outer(inv_freq, t).astype(np.float32)    # [64, S]
    cosT = np.concatenate([np.cos(freqs), np.cos(freqs)], 0).astype(np.float32)
    sinS = np.concatenate([-np.sin(freqs), np.sin(freqs)], 0).astype(np.float32)

    # shifted tril mask bank: tri[p, x] = 1 if p <= x - 384
    p = np.arange(128)[:, None]
    xx = np.arange(896)[None, :]
    tri = (p <= xx - 384).astype(ml_dtypes.bfloat16)

    esc = np.full((128, 1), s_p * s_p / np.sqrt(np.float32(D)), np.float32)
    osc = np.full((128, 1), s_p * s_o, np.float32)

    in_maps = []
    for c in range(NCORES):
        r = slice(c * HPC * D, (c + 1) * HPC * D)       # 256 features
        wt_c = np.ascontiguousarray(
            np.concatenate([tp[:H][r], tp[H:2 * H][r], tp[2 * H:][r]], 0).T
        ).astype(ml_dtypes.bfloat16)
        wot_c = np.ascontiguousarray(to[:, r].T).astype(ml_dtypes.bfloat16)
        in_maps.append({
            "xt": xt, "wt": wt_c, "wot": wot_c, "cost": cosT, "sins": sinS,
            "tri": tri, "osq": np.ones((128, 128), ml_dtypes.bfloat16),
            "esc": esc, "osc": osc,
        })
    return in_maps


def kernel(hidden_states, attention_mask, w_proj, w_o):
    global _built
    if _built is None:
        _built = _build()
    nc = _built
    in_maps = _host_prep(hidden_states, w_proj, w_o)
    res = run_bass_kernel_spmd(nc, in_maps, core_ids=list(range(NCORES)))
    acc = np.zeros((S, H), np.float32)
    for c in range(NCORES):
        acc += res.results[c]["out"].astype(np.float32)
    return acc.reshape(1, S, H)


# revision 29
# speedup vs baseline: 1.2872x; 1.0563x over previous
"""BitNet attention layer on 8 Trainium2 NeuronCores.

Tensor-parallel over heads: core i owns heads {2i, 2i+1}. Each core:
  - computes q^T,k^T (feature-major) + v (natural) for its heads via bf16
    matmuls against host-pretransposed x^T and ternary-quantized W^T slices
  - RoPE on q^T/k^T (partition-dim rotate-half, sign folded into sin table)
  - causal attention with transposed scores S^T[k,q]; diagonal 128-blocks use
    triangular free-dim slices so fully-masked columns are never computed;
    softmax denominator via elementwise chunk pre-sum on DVE + a single
    ones-matmul partition reduce per (tile, head)
  - o_proj partial over its 256 ctx features -> fp16 partial [2048, 2048]
Host sums the 8 partials.

All matmuls run with bf16 operands (1 cycle/row on the PE at any free size).
DMA: weight and x^T tiles stream interleaved so the first projection chain
starts ~5us in; the first seq-tile consumes h-chunk groups as they arrive.
"""
import os
import sys

import numpy as np

try:
    import concourse.bass as bass
except ImportError:
    sys.path.insert(0, "/opt/trn_rl_repo")
    import concourse.bass as bass

import concourse.mybir as mybir
import concourse.tile as tile
from concourse import bacc
from concourse.bass_utils import run_bass_kernel_spmd

F32 = mybir.dt.float32
F32R = mybir.dt.float32r
F16 = mybir.dt.float16
BF16 = mybir.dt.bfloat16

S = 2048          # sequence length
H = 2048          # hidden
D = 128           # head dim
NCORES = 8
HPC = 2           # heads per core
OC = 3 * HPC * D  # 768 per-core projection output features (q|k|v)
ST = 512          # seq tile for projection rhs / attention qi tile
NST = S // ST     # 4
HC = H // 128     # 16 h-chunks
HG = 2            # h-chunk group size (DMA granularity)
NG = HC // HG     # 8 groups
ROPE_BASE = 10000.0

_built = None
_PHASES = os.environ.get("KPH", "ABC")


def _build(timing=False):
    nc = bacc.Bacc("TRN2", target_bir_lowering=False, debug=False,
                   dynamic_dma_scratch_size=4096)

    if timing:
        # timing variant: identical device work, but big tensors live in
        # internal DRAM (garbage data) so per-call host<->device transfer is
        # tiny and wall-clock deltas measure the NEFF itself.
        xt_d = nc.dram_tensor("xt_i", [H, S], BF16)
        wt_d = nc.dram_tensor("wt_i", [H, OC], BF16)
        wot_d = nc.dram_tensor("wot_i", [HPC * D, H], BF16)
        cos_d = nc.dram_tensor("cost_i", [D, S], F32)
        sin_d = nc.dram_tensor("sins_i", [D, S], F32)
        tri_d = nc.dram_tensor("tri_i", [128, 896], BF16)
        out_d = nc.dram_tensor("out_i", [S, H], F16)
        out_x = nc.declare_dram_parameter("out", [128, H], F16, isOutput=True)
    else:
        xt_d = nc.declare_dram_parameter("xt", [H, S], BF16, isOutput=False)
        wt_d = nc.declare_dram_parameter("wt", [H, OC], BF16, isOutput=False)
        wot_d = nc.declare_dram_parameter("wot", [HPC * D, H], BF16,
                                          isOutput=False)
        cos_d = nc.declare_dram_parameter("cost", [D, S], F32, isOutput=False)
        sin_d = nc.declare_dram_parameter("sins", [D, S], F32, isOutput=False)
        tri_d = nc.declare_dram_parameter("tri", [128, 896], BF16,
                                          isOutput=False)
        out_d = nc.declare_dram_parameter("out", [S, H], F16, isOutput=True)
    osq_d = nc.declare_dram_parameter("osq", [128, 128], BF16, isOutput=False)

    # exp scale (s_p^2/sqrt(D)) and output scale (s_p*s_o) are runtime values;
    # pass them as tiny per-partition inputs instead of baking into the NEFF.
    esc_d = nc.declare_dram_parameter("esc", [128, 1], F32, isOutput=False)
    osc_d = nc.declare_dram_parameter("osc", [128, 1], F32, isOutput=False)

    with tile.TileContext(nc) as tc, nc.allow_low_precision(
        reason="bf16 matmul operands / probs; validated 3.3e-3 rel err"
    ):
        with tc.tile_pool(name="const", bufs=1) as cpool, \
             tc.tile_pool(name="qkv", bufs=1) as qpool, \
             tc.tile_pool(name="ctx", bufs=1) as xpool, \
             tc.tile_pool(name="wo", bufs=1) as wopool, \
             tc.tile_pool(name="ob", bufs=5) as opool:
            cost = cpool.tile([D, S], F32)
            sins = cpool.tile([D, S], F32)
            tri = cpool.tile([128, 896], BF16)
            osq = cpool.tile([128, 128], BF16)
            esc = cpool.tile([128, 1], F32)
            osc = cpool.tile([128, 1], F32)
            wot = wopool.tile([128, HPC, H], BF16)

            # persistent per-head tensors, tiled per seq-tile for fine deps
            qk = [[qpool.tile([D, ST], BF16, name=f"qk{oc}_{st}")
                   for st in range(NST)] for oc in range(4)]
            v_sb = [qpool.tile([128, ST // 128, HPC * D], BF16, name=f"v{st}")
                    for st in range(NST)]
            ctx = [[xpool.tile([D, ST], BF16, name=f"ctx{h}_{t}")
                    for t in range(NST)] for h in range(HPC)]

            # ---------------- Phase A: qkv projection + RoPE ----------------
            if "A" in _PHASES:
             with tc.tile_pool(name="wt", bufs=1) as wpool, \
                 tc.tile_pool(name="xt", bufs=2) as xtpool, \
                 tc.tile_pool(name="ropet", bufs=2) as rpool, \
                 tc.tile_pool(name="psA", bufs=4, space="PSUM") as psA, \
                 tc.tile_pool(name="psV", bufs=2, space="PSUM") as psV:
                wt = [wpool.tile([128, HG, OC], BF16, name=f"wt{g}")
                      for g in range(NG)]
                xt0 = [xtpool.tile([128, HG, ST], BF16, name=f"xt{g}")
                       for g in range(NG)]
                # critical startup stream: weight group then matching x group
                for g in range(NG):
                    nc.sync.dma_start(
                        wt[g][:],
                        wt_d[g * HG * 128:(g + 1) * HG * 128].rearrange(
                            "(ho hp) o -> hp ho o", hp=128))
                    nc.sync.dma_start(
                        xt0[g][:],
                        xt_d[g * HG * 128:(g + 1) * HG * 128, 0:ST].rearrange(
                            "(ho hp) s -> hp ho s", hp=128))
                # non-critical constants after the startup stream
                nc.sync.dma_start(cost[:], cos_d[:])
                nc.sync.dma_start(sins[:], sin_d[:])
                nc.sync.dma_start(tri[:], tri_d[:])
                nc.sync.dma_start(
                    wot[:],
                    wot_d.rearrange("(co cp) o -> cp co o", cp=128))
                nc.sync.dma_start(esc[:], esc_d[:])
                nc.sync.dma_start(osc[:], osc_d[:])
                nc.sync.dma_start(osq[:], osq_d[:])

                def rope(dst, ps, ssl):
                    t2 = rpool.tile([128, ST], BF16, name="t2")
                    nc.vector.tensor_mul(t2[0:64, :], ps[64:128, :],
                                         sins[0:64, ssl])
                    nc.vector.tensor_mul(t2[64:128, :], ps[0:64, :],
                                         sins[64:128, ssl])
                    nc.vector.tensor_mul(dst[:], ps[:], cost[:, ssl])
                    nc.vector.tensor_add(dst[:], dst[:], t2[:])

                def v_chain(st, xt, sc):
                    ps = psV.tile([128, HPC * D], F32)
                    for hcc in range(HC):
                        nc.tensor.matmul(
                            ps[:],
                            xt[hcc // HG][:, hcc % HG,
                                          sc * 128:(sc + 1) * 128],
                            wt[hcc // HG][:, hcc % HG, 4 * 128:],
                            start=(hcc == 0), stop=(hcc == HC - 1))
                    nc.scalar.copy(v_sb[st][:, sc, :], ps[:])

                def v_pass(st, xt):
                    for sc in range(ST // 128):
                        v_chain(st, xt, sc)

                # st0: h-chunk-major so the PE consumes groups as they arrive
                psA0 = [psA.tile([128, ST], F32, name="ps") for oc in range(4)]
                for g in range(NG):
                    for hq in range(HG):
                        hcc = g * HG + hq
                        for oc in range(4):
                            nc.tensor.matmul(
                                psA0[oc][:],
                                wt[g][:, hq, oc * 128:(oc + 1) * 128],
                                xt0[g][:, hq, :],
                                start=(hcc == 0), stop=(hcc == HC - 1))
                for oc in range(4):
                    rope(qk[oc][0], psA0[oc], slice(0, ST))
                v_pass(0, xt0)

                # st1..3: oc-major chains, xt double-buffered
                for st in range(1, NST):
                    ssl = slice(st * ST, (st + 1) * ST)
                    xt = [xtpool.tile([128, HG, ST], BF16, name=f"xt{g}")
                          for g in range(NG)]
                    for g in range(NG):
                        nc.sync.dma_start(
                            xt[g][:],
                            xt_d[g * HG * 128:(g + 1) * HG * 128, ssl].rearrange(
                                "(ho hp) s -> hp ho s", hp=128))
                    for oc in range(4):
                        ps = psA.tile([128, ST], F32, name="ps")
                        for hcc in range(HC):
                            nc.tensor.matmul(
                                ps[:],
                                wt[hcc // HG][:, hcc % HG,
                                              oc * 128:(oc + 1) * 128],
                                xt[hcc // HG][:, hcc % HG, :],
                                start=(hcc == 0), stop=(hcc == HC - 1))
                        rope(qk[oc][st], ps, ssl)
                    v_pass(st, xt)

            # ---------- Phase B+C: attention + o_proj, interleaved ----------
            def copy_scaled(engine, dst, src):
                if engine == "act":
                    nc.scalar.activation(
                        dst, src, mybir.ActivationFunctionType.Copy,
                        bias=0.0, scale=osc[:])
                elif engine == "dve":
                    nc.vector.tensor_scalar_mul(dst, src, osc[:])
                else:
                    nc.gpsimd.tensor_scalar_mul(dst, src, osc[:])

            def oproj_ob(t, sc, half, psO_, engines, ei, fused):
                """One [128, H/2] output block: two psum chains, head-0 matmul
                first in each chain so the PE can proceed while head-1's ctx
                normalize drains; psum->sbuf copies round-robin `engines`.
                fused: both chains in one 2-bank psum tile, one [128,1024]
                copy."""
                ob = opool.tile([128, H // 2], F16)
                if fused:
                    po = psO_.tile([128, 2, ST], F32, name="po")
                    for cc in range(HPC):
                        for oth in range(2):
                            ot = half * 2 + oth
                            nc.tensor.matmul(
                                po[:, oth, :],
                                ctx[cc][t][:, (sc % 4) * 128:
                                           (sc % 4 + 1) * 128],
                                wot[:, cc, ot * ST:(ot + 1) * ST],
                                start=(cc == 0), stop=(cc == HPC - 1))
                    copy_scaled(engines[ei % len(engines)], ob[:], po[:])
                else:
                    for oth in range(2):
                        ot = half * 2 + oth
                        po1 = psO_.tile([128, ST], F32, name="po")
                        for cc in range(HPC):
                            nc.tensor.matmul(
                                po1[:],
                                ctx[cc][t][:, (sc % 4) * 128:
                                           (sc % 4 + 1) * 128],
                                wot[:, cc, ot * ST:(ot + 1) * ST],
                                start=(cc == 0), stop=(cc == HPC - 1))
                        copy_scaled(engines[(ei + oth) % len(engines)],
                                    ob[:, oth * ST:(oth + 1) * ST], po1[:])
                nc.sync.dma_start(
                    out_d[sc * 128:(sc + 1) * 128,
                          half * (H // 2):(half + 1) * (H // 2)], ob[:])

            def oproj(t, psO_, engines, fused=False, skip=0):
                ei = 0
                for sc in range(4 * t, 4 * t + 4):
                    for half in range(2):
                        if ei >= skip:
                            oproj_ob(t, sc, half, psO_, engines, ei, fused)
                        ei += 1

            if "B" in _PHASES:
             with tc.tile_pool(name="pt", bufs=1) as ptpool, \
                 tc.tile_pool(name="rden", bufs=2) as dpool, \
                 tc.tile_pool(name="ptsum", bufs=2) as spool, \
                 tc.tile_pool(name="psO", bufs=2, space="PSUM") as psO, \
                 tc.tile_pool(name="psS", bufs=3, space="PSUM") as psS, \
                 tc.tile_pool(name="psB", bufs=1, space="PSUM") as psB, \
                 tc.tile_pool(name="psC", bufs=2, space="PSUM") as psC:
                for t in range(NST):
                    nkj = 4 * (t + 1)
                    for h in range(HPC):
                        pt = ptpool.tile([128, nkj, ST], BF16, name=f"pt{h}")
                        ptsum = spool.tile([128, ST], BF16, name="ptsum")
                        # scores S^T[kj, qi] per 128-chunk; diagonal chunks
                        # only compute the un-masked qi suffix [128i:]
                        for j in range(nkj):
                            di = j - 4 * t       # >= 0 on diagonal chunks
                            lo = 128 * di if di >= 0 else 0
                            sp = psS.tile([128, ST], F32, name="sp")
                            nc.tensor.matmul(
                                sp[:, lo:],
                                qk[2 + h][j // 4][:, (j % 4) * 128:
                                                  (j % 4 + 1) * 128],
                                qk[h][t][:, lo:],
                                start=True, stop=True)
                            # probs (unnormalized): exp(esc * scores)
                            nc.scalar.activation(
                                pt[:, j, lo:], sp[:, lo:],
                                mybir.ActivationFunctionType.Exp,
                                bias=0.0, scale=esc[:])
                            if di >= 0:  # diagonal block: tril mask
                                nc.vector.tensor_mul(
                                    pt[:, j, lo:], pt[:, j, lo:],
                                    tri[:, 384:896 - lo])
                            # elementwise chunk pre-sum for the softmax
                            # denominator (partition reduce happens once via
                            # the ones-matmul below)
                            if j == 0:
                                nc.vector.tensor_copy(ptsum[:], pt[:, 0, :])
                            else:
                                nc.vector.tensor_add(
                                    ptsum[:, lo:], ptsum[:, lo:],
                                    pt[:, j, lo:])
                        # ctx^T[d, qi] accumulate over kj; the denominator
                        # ones-matmul + reciprocal are emitted mid-chain so
                        # rbp is ready before cp completes and the final
                        # normalize costs a single DVE mul
                        cp = psC.tile([128, ST], F32, name="cp")
                        bp = psB.tile([128, ST], F32)
                        rbp = dpool.tile([128, ST], F32, name="rbp")
                        for j in range(nkj):
                            di = j - 4 * t
                            lo = 128 * di if di >= 0 else 0
                            nc.tensor.matmul(
                                cp[:, lo:],
                                v_sb[j // 4][:, j % 4, h * D:(h + 1) * D],
                                pt[:, j, lo:],
                                start=(j == 0), stop=(j == nkj - 1))
                            if j == nkj - 2:
                                # ptsum complete by now (trails exps)
                                nc.tensor.matmul(bp[:], osq[:], ptsum[:],
                                                 start=True, stop=True)
                                nc.vector.reciprocal(rbp[:], bp[:])
                        nc.vector.tensor_mul(ctx[h][t][:], cp[:], rbp[:])

                    if "C" in _PHASES:
                        if t < NST - 1:
                            oproj(t, psO, ["act", "dve"])
                        else:
                            # first two output blocks through the shared pool:
                            # they start while the psO3 scope below waits for
                            # the attention pools' psum banks to free
                            oproj_ob(t, 4 * t, 0, psO, ["act", "dve"], 0,
                                     fused=False)
                            oproj_ob(t, 4 * t, 1, psO, ["act", "dve"], 1,
                                     fused=False)

            # rest of the last tile's o_proj: nothing left to interleave, so
            # deep fused psum buffering, copies ending on the faster ACT
            if "B" in _PHASES and "C" in _PHASES:
                with tc.tile_pool(name="psO3", bufs=3, space="PSUM") as psO3:
                    oproj(NST - 1, psO3, ["dve", "act"], fused=True, skip=2)

            if timing:
                nc.sync.dma_start(out_x[:], out_d[S - 128:, :])

    nc.compile()
    return nc


def _host_prep(hidden_states, w_proj, w_o):
    import ml_dtypes
    x = np.asarray(hidden_states, dtype=np.float32).reshape(S, H)
    w_proj = np.asarray(w_proj, dtype=np.float32)
    w_o = np.asarray(w_o, dtype=np.float32)

    # BitNet b1.58 per-tensor absmean quantization (ternary, scale factored out)
    s_p = np.float32(np.mean(np.abs(w_proj), dtype=np.float32)) + np.float32(1e-5)
    s_o = np.float32(np.mean(np.abs(w_o), dtype=np.float32)) + np.float32(1e-5)
    tp = np.clip(np.round(w_proj / s_p), -1.0, 1.0).astype(np.float32)
    to = np.clip(np.round(w_o / s_o), -1.0, 1.0).astype(np.float32)

    xt = np.ascontiguousarray(x.T).astype(ml_dtypes.bfloat16)   # [H, S]

    # RoPE tables, feature-major, rotate-half sign folded into sin
    inv_freq = (1.0 / (ROPE_BASE ** (np.arange(0, D, 2, dtype=np.float32) / D))
                ).astype(np.float32)
    t = np.arange(S, dtype=np.float32)
    freqs = np.outer(inv_freq, t).astype(np.float32)    # [64, S]
    cosT = np.concatenate([np.cos(freqs), np.cos(freqs)], 0).astype(np.float32)
    sinS = np.concatenate([-np.sin(freqs), np.sin(freqs)], 0).astype(np.float32)

    # shifted tril mask bank: tri[p, x] = 1 if p <= x - 384
    p = np.arange(128)[:, None]
    xx = np.arange(896)[None, :]
    tri = (p <= xx - 384).astype(ml_dtypes.bfloat16)

    esc = np.full((128, 1), s_p * s_p / np.sqrt(np.float32(D)), np.float32)
    osc = np.full((128, 1), s_p * s_o, np.float32)

    in_maps = []
    for c in range(NCORES):
        r = slice(c * HPC * D, (c + 1) * HPC * D)       # 256 features
        wt_c = np.ascontiguousarray(
            np.concatenate([tp[:H][r], tp[H:2 * H][r], tp[2 * H:][r]], 0).T
        ).astype(ml_dtypes.bfloat16)
        wot_c = np.ascontiguousarray(to[:, r].T).astype(ml_dtypes.bfloat16)
        in_maps.append({
            "xt": xt, "wt": wt_c, "wot": wot_c, "cost": cosT, "sins": sinS,
            "tri": tri, "osq": np.ones((128, 128), ml_dtypes.bfloat16),
            "esc": esc, "osc": osc,
        })
    return in_maps


def kernel(hidden_states, attention_mask, w_proj, w_o):
    global _built
    if _built is None:
        _built = _build()
    nc = _built
    in_maps = _host_prep(hidden_states, w_proj, w_o)
    res = run_bass_kernel_spmd(nc, in_maps, core_ids=list(range(NCORES)))
    acc = np.zeros((S, H), np.float32)
    for c in range(NCORES):
        acc += res.results[c]["out"].astype(np.float32)
    return acc.reshape(1, S, H)


# revision 35
# speedup vs baseline: 1.3133x; 1.0203x over previous
"""BitNet attention layer on 8 Trainium2 NeuronCores.

Tensor-parallel over heads: core i owns heads {2i, 2i+1}. Each core:
  - computes q^T,k^T (feature-major) + v (natural) for its heads via bf16
    matmuls against host-pretransposed x^T and ternary-quantized W^T slices
  - RoPE on q^T/k^T (partition-dim rotate-half, sign folded into sin table)
  - causal attention with transposed scores S^T[k,q]; diagonal 128-blocks use
    triangular free-dim slices so fully-masked columns are never computed;
    softmax denominator via elementwise chunk pre-sum on DVE + a single
    ones-matmul partition reduce per (tile, head)
  - o_proj partial over its 256 ctx features -> fp16 partial [2048, 2048]
Host sums the 8 partials.

All matmuls run with bf16 operands (1 cycle/row on the PE at any free size).
DMA: weight and x^T tiles stream interleaved so the first projection chain
starts ~5us in; the first seq-tile consumes h-chunk groups as they arrive.
"""
import os
import sys

import numpy as np

try:
    import concourse.bass as bass
except ImportError:
    sys.path.insert(0, "/opt/trn_rl_repo")
    import concourse.bass as bass

import concourse.mybir as mybir
import concourse.tile as tile
from concourse import bacc
from concourse.bass_utils import run_bass_kernel_spmd

F32 = mybir.dt.float32
F32R = mybir.dt.float32r
F16 = mybir.dt.float16
BF16 = mybir.dt.bfloat16

S = 2048          # sequence length
H = 2048          # hidden
D = 128           # head dim
NCORES = 8
HPC = 2           # heads per core
OC = 3 * HPC * D  # 768 per-core projection output features (q|k|v)
ST = 512          # seq tile for projection rhs / attention qi tile
NST = S // ST     # 4
HC = H // 128     # 16 h-chunks
HG = 2            # h-chunk group size (DMA granularity)
NG = HC // HG     # 8 groups
ROPE_BASE = 10000.0

_built = None
_PHASES = os.environ.get("KPH", "ABC")


def _build(timing=False):
    nc = bacc.Bacc("TRN2", target_bir_lowering=False, debug=False,
                   dynamic_dma_scratch_size=4096)

    if timing:
        # timing variant: identical device work, but big tensors live in
        # internal DRAM (garbage data) so per-call host<->device transfer is
        # tiny and wall-clock deltas measure the NEFF itself.
        xt_d = nc.dram_tensor("xt_i", [H, S], BF16)
        wt_d = nc.dram_tensor("wt_i", [H, OC], BF16)
        wot_d = nc.dram_tensor("wot_i", [HPC * D, H], BF16)
        cos_d = nc.dram_tensor("cost_i", [D, S], BF16)
        sin_d = nc.dram_tensor("sins_i", [D, S], BF16)
        tri_d = nc.dram_tensor("tri_i", [128, 896], BF16)
        out_d = nc.dram_tensor("out_i", [S, H], F16)
        out_x = nc.declare_dram_parameter("out", [128, H], F16, isOutput=True)
    else:
        xt_d = nc.declare_dram_parameter("xt", [H, S], BF16, isOutput=False)
        wt_d = nc.declare_dram_parameter("wt", [H, OC], BF16, isOutput=False)
        wot_d = nc.declare_dram_parameter("wot", [HPC * D, H], BF16,
                                          isOutput=False)
        cos_d = nc.declare_dram_parameter("cost", [D, S], BF16, isOutput=False)
        sin_d = nc.declare_dram_parameter("sins", [D, S], BF16, isOutput=False)
        tri_d = nc.declare_dram_parameter("tri", [128, 896], BF16,
                                          isOutput=False)
        out_d = nc.declare_dram_parameter("out", [S, H], F16, isOutput=True)
    osq_d = nc.declare_dram_parameter("osq", [128, 128], BF16, isOutput=False)

    # exp scale (s_p^2/sqrt(D)) and output scale (s_p*s_o) are runtime values;
    # pass them as tiny per-partition inputs instead of baking into the NEFF.
    esc_d = nc.declare_dram_parameter("esc", [128, 1], F32, isOutput=False)
    osc_d = nc.declare_dram_parameter("osc", [128, 1], F32, isOutput=False)

    with tile.TileContext(nc) as tc, nc.allow_low_precision(
        reason="bf16 matmul operands / probs; validated 3.3e-3 rel err"
    ):
        with tc.tile_pool(name="const", bufs=1) as cpool, \
             tc.tile_pool(name="qkv", bufs=1) as qpool, \
             tc.tile_pool(name="ctx", bufs=1) as xpool, \
             tc.tile_pool(name="wo", bufs=1) as wopool, \
             tc.tile_pool(name="ob", bufs=5) as opool:
            cost = cpool.tile([D, S], BF16)
            sins = cpool.tile([D, S], BF16)
            tri = cpool.tile([128, 896], BF16)
            osq = cpool.tile([128, 128], BF16)
            esc = cpool.tile([128, 1], F32)
            osc = cpool.tile([128, 1], F32)
            wot = wopool.tile([128, HPC, H], BF16)

            # persistent per-head tensors, tiled per seq-tile for fine deps
            qk = [[qpool.tile([D, ST], BF16, name=f"qk{oc}_{st}")
                   for st in range(NST)] for oc in range(4)]
            v_sb = [qpool.tile([128, ST // 128, HPC * D], BF16, name=f"v{st}")
                    for st in range(NST)]
            ctx = [[xpool.tile([D, ST], BF16, name=f"ctx{h}_{t}")
                    for t in range(NST)] for h in range(HPC)]

            # ---------------- Phase A: qkv projection + RoPE ----------------
            if "A" in _PHASES:
             with tc.tile_pool(name="wt", bufs=1) as wpool, \
                 tc.tile_pool(name="xt", bufs=2) as xtpool, \
                 tc.tile_pool(name="ropet", bufs=2) as rpool, \
                 tc.tile_pool(name="psA", bufs=4, space="PSUM") as psA, \
                 tc.tile_pool(name="psV", bufs=2, space="PSUM") as psV:
                wt = [wpool.tile([128, HG, OC], BF16, name=f"wt{g}")
                      for g in range(NG)]
                xt0 = [xtpool.tile([128, HG, ST], BF16, name=f"xt{g}")
                       for g in range(NG)]
                # warm up the PE p-state during the initial DMA wait: osq is
                # tiny (32KB, arrives in ~1us) and the spam matmuls keep the
                # tensor clock ramping while wt/xt stream in
                nc.sync.dma_start(osq[:], osq_d[:])
                wps = psA.tile([128, ST], F32, name="ps")
                for _ in range(40):
                    nc.tensor.matmul(wps[:, 0:128], osq[:], osq[:],
                                     start=True, stop=True)
                # critical startup stream: weight group then matching x group
                for g in range(NG):
                    nc.sync.dma_start(
                        wt[g][:],
                        wt_d[g * HG * 128:(g + 1) * HG * 128].rearrange(
                            "(ho hp) o -> hp ho o", hp=128))
                    nc.sync.dma_start(
                        xt0[g][:],
                        xt_d[g * HG * 128:(g + 1) * HG * 128, 0:ST].rearrange(
                            "(ho hp) s -> hp ho s", hp=128))
                # rope tables next (needed at st0's end); heavier non-critical
                # constants are deferred behind st2's xt stream
                nc.sync.dma_start(cost[:], cos_d[:])
                nc.sync.dma_start(sins[:], sin_d[:])
                nc.sync.dma_start(tri[:], tri_d[:])

                def rope(dst, ps, ssl):
                    t2 = rpool.tile([128, ST], BF16, name="t2")
                    nc.vector.tensor_mul(t2[0:64, :], ps[64:128, :],
                                         sins[0:64, ssl])
                    nc.vector.tensor_mul(t2[64:128, :], ps[0:64, :],
                                         sins[64:128, ssl])
                    nc.vector.tensor_mul(dst[:], ps[:], cost[:, ssl])
                    nc.vector.tensor_add(dst[:], dst[:], t2[:])

                def v_chain(st, xt, sc):
                    ps = psV.tile([128, HPC * D], F32)
                    for hcc in range(HC):
                        nc.tensor.matmul(
                            ps[:],
                            xt[hcc // HG][:, hcc % HG,
                                          sc * 128:(sc + 1) * 128],
                            wt[hcc // HG][:, hcc % HG, 4 * 128:],
                            start=(hcc == 0), stop=(hcc == HC - 1))
                    nc.scalar.copy(v_sb[st][:, sc, :], ps[:])

                def v_pass(st, xt):
                    for sc in range(ST // 128):
                        v_chain(st, xt, sc)

                # st0: h-chunk-major so the PE consumes groups as they arrive
                psA0 = [psA.tile([128, ST], F32, name="ps") for oc in range(4)]
                for g in range(NG):
                    for hq in range(HG):
                        hcc = g * HG + hq
                        for oc in range(4):
                            nc.tensor.matmul(
                                psA0[oc][:],
                                wt[g][:, hq, oc * 128:(oc + 1) * 128],
                                xt0[g][:, hq, :],
                                start=(hcc == 0), stop=(hcc == HC - 1))
                for oc in range(4):
                    rope(qk[oc][0], psA0[oc], slice(0, ST))
                v_pass(0, xt0)

                # st1..3: oc-major chains, xt double-buffered
                for st in range(1, NST):
                    ssl = slice(st * ST, (st + 1) * ST)
                    xt = [xtpool.tile([128, HG, ST], BF16, name=f"xt{g}")
                          for g in range(NG)]
                    for g in range(NG):
                        nc.sync.dma_start(
                            xt[g][:],
                            xt_d[g * HG * 128:(g + 1) * HG * 128, ssl].rearrange(
                                "(ho hp) s -> hp ho s", hp=128))
                    if st == 2:
                        nc.sync.dma_start(
                            wot[:],
                            wot_d.rearrange("(co cp) o -> cp co o", cp=128))
                        nc.sync.dma_start(esc[:], esc_d[:])
                        nc.sync.dma_start(osc[:], osc_d[:])
                    for oc in range(4):
                        ps = psA.tile([128, ST], F32, name="ps")
                        for hcc in range(HC):
                            nc.tensor.matmul(
                                ps[:],
                                wt[hcc // HG][:, hcc % HG,
                                              oc * 128:(oc + 1) * 128],
                                xt[hcc // HG][:, hcc % HG, :],
                                start=(hcc == 0), stop=(hcc == HC - 1))
                        rope(qk[oc][st], ps, ssl)
                    v_pass(st, xt)

            # ---------- Phase B+C: attention + o_proj, interleaved ----------
            def copy_scaled(engine, dst, src):
                if engine == "act":
                    nc.scalar.activation(
                        dst, src, mybir.ActivationFunctionType.Copy,
                        bias=0.0, scale=osc[:])
                elif engine == "dve":
                    nc.vector.tensor_scalar_mul(dst, src, osc[:])
                else:
                    nc.gpsimd.tensor_scalar_mul(dst, src, osc[:])

            def oproj_ob(t, sc, half, psO_, engines, ei, fused):
                """One [128, H/2] output block: two psum chains, head-0 matmul
                first in each chain so the PE can proceed while head-1's ctx
                normalize drains; psum->sbuf copies round-robin `engines`.
                fused: both chains in one 2-bank psum tile, one [128,1024]
                copy."""
                ob = opool.tile([128, H // 2], F16)
                if fused:
                    po = psO_.tile([128, 2, ST], F32, name="po")
                    for cc in range(HPC):
                        for oth in range(2):
                            ot = half * 2 + oth
                            nc.tensor.matmul(
                                po[:, oth, :],
                                ctx[cc][t][:, (sc % 4) * 128:
                                           (sc % 4 + 1) * 128],
                                wot[:, cc, ot * ST:(ot + 1) * ST],
                                start=(cc == 0), stop=(cc == HPC - 1))
                    copy_scaled(engines[ei % len(engines)], ob[:], po[:])
                else:
                    for oth in range(2):
                        ot = half * 2 + oth
                        po1 = psO_.tile([128, ST], F32, name="po")
                        for cc in range(HPC):
                            nc.tensor.matmul(
                                po1[:],
                                ctx[cc][t][:, (sc % 4) * 128:
                                           (sc % 4 + 1) * 128],
                                wot[:, cc, ot * ST:(ot + 1) * ST],
                                start=(cc == 0), stop=(cc == HPC - 1))
                        copy_scaled(engines[(ei + oth) % len(engines)],
                                    ob[:, oth * ST:(oth + 1) * ST], po1[:])
                nc.sync.dma_start(
                    out_d[sc * 128:(sc + 1) * 128,
                          half * (H // 2):(half + 1) * (H // 2)], ob[:])

            def oproj(t, psO_, engines, fused=False, skip=0):
                ei = 0
                for sc in range(4 * t, 4 * t + 4):
                    for half in range(2):
                        if ei >= skip:
                            oproj_ob(t, sc, half, psO_, engines, ei, fused)
                        ei += 1

            if "B" in _PHASES:
             with tc.tile_pool(name="pt", bufs=1) as ptpool, \
                 tc.tile_pool(name="rden", bufs=2) as dpool, \
                 tc.tile_pool(name="ptsum", bufs=2) as spool, \
                 tc.tile_pool(name="psO", bufs=2, space="PSUM") as psO, \
                 tc.tile_pool(name="psS", bufs=3, space="PSUM") as psS, \
                 tc.tile_pool(name="psB", bufs=1, space="PSUM") as psB, \
                 tc.tile_pool(name="psC", bufs=2, space="PSUM") as psC:
                for t in range(NST):
                    nkj = 4 * (t + 1)
                    for h in range(HPC):
                        pt = ptpool.tile([128, nkj, ST], BF16, name=f"pt{h}")
                        ptsum = spool.tile([128, ST], BF16, name="ptsum")
                        # scores S^T[kj, qi] per 128-chunk; diagonal chunks
                        # only compute the un-masked qi suffix [128i:]
                        for j in range(nkj):
                            di = j - 4 * t       # >= 0 on diagonal chunks
                            lo = 128 * di if di >= 0 else 0
                            sp = psS.tile([128, ST], F32, name="sp")
                            nc.tensor.matmul(
                                sp[:, lo:],
                                qk[2 + h][j // 4][:, (j % 4) * 128:
                                                  (j % 4 + 1) * 128],
                                qk[h][t][:, lo:],
                                start=True, stop=True)
                            # probs (unnormalized): exp(esc * scores)
                            nc.scalar.activation(
                                pt[:, j, lo:], sp[:, lo:],
                                mybir.ActivationFunctionType.Exp,
                                bias=0.0, scale=esc[:])
                            if di >= 0:  # diagonal block: tril mask
                                nc.vector.tensor_mul(
                                    pt[:, j, lo:], pt[:, j, lo:],
                                    tri[:, 384:896 - lo])
                            # elementwise chunk pre-sum for the softmax
                            # denominator (partition reduce happens once via
                            # the ones-matmul below)
                            if j == 0:
                                nc.vector.tensor_copy(ptsum[:], pt[:, 0, :])
                            else:
                                nc.vector.tensor_add(
                                    ptsum[:, lo:], ptsum[:, lo:],
                                    pt[:, j, lo:])
                        # ctx^T[d, qi] accumulate over kj; the denominator
                        # ones-matmul + reciprocal are emitted mid-chain so
                        # rbp is ready before cp completes and the final
                        # normalize costs a single DVE mul
                        cp = psC.tile([128, ST], F32, name="cp")
                        bp = psB.tile([128, ST], F32)
                        rbp = dpool.tile([128, ST], F32, name="rbp")
                        for j in range(nkj):
                            di = j - 4 * t
                            lo = 128 * di if di >= 0 else 0
                            nc.tensor.matmul(
                                cp[:, lo:],
                                v_sb[j // 4][:, j % 4, h * D:(h + 1) * D],
                                pt[:, j, lo:],
                                start=(j == 0), stop=(j == nkj - 1))
                            if j == nkj - 2:
                                # ptsum complete by now (trails exps)
                                nc.tensor.matmul(bp[:], osq[:], ptsum[:],
                                                 start=True, stop=True)
                                nc.vector.reciprocal(rbp[:], bp[:])
                        nc.vector.tensor_mul(ctx[h][t][:], cp[:], rbp[:])

                    if "C" in _PHASES:
                        if t < NST - 1:
                            oproj(t, psO, ["act", "dve"])
                        else:
                            # first two output blocks through the shared pool:
                            # they start while the psO3 scope below waits for
                            # the attention pools' psum banks to free
                            oproj_ob(t, 4 * t, 0, psO, ["act", "dve"], 0,
                                     fused=False)
                            oproj_ob(t, 4 * t, 1, psO, ["act", "dve"], 1,
                                     fused=False)

            # rest of the last tile's o_proj: nothing left to interleave, so
            # deep fused psum buffering, copies ending on the faster ACT
            if "B" in _PHASES and "C" in _PHASES:
                with tc.tile_pool(name="psO3", bufs=3, space="PSUM") as psO3:
                    oproj(NST - 1, psO3, ["dve", "act"], fused=True, skip=2)

            if timing:
                nc.sync.dma_start(out_x[:], out_d[S - 128:, :])

    nc.compile()
    return nc


def _host_prep(hidden_states, w_proj, w_o):
    import ml_dtypes
    x = np.asarray(hidden_states, dtype=np.float32).reshape(S, H)
    w_proj = np.asarray(w_proj, dtype=np.float32)
    w_o = np.asarray(w_o, dtype=np.float32)

    # BitNet b1.58 per-tensor absmean quantization (ternary, scale factored out)
    s_p = np.float32(np.mean(np.abs(w_proj), dtype=np.float32)) + np.float32(1e-5)
    s_o = np.float32(np.mean(np.abs(w_o), dtype=np.float32)) + np.float32(1e-5)
    tp = np.clip(np.round(w_proj / s_p), -1.0, 1.0).astype(np.float32)
    to = np.clip(np.round(w_o / s_o), -1.0, 1.0).astype(np.float32)

    xt = np.ascontiguousarray(x.T).astype(ml_dtypes.bfloat16)   # [H, S]

    # RoPE tables, feature-major, rotate-half sign folded into sin
    inv_freq = (1.0 / (ROPE_BASE ** (np.arange(0, D, 2, dtype=np.float32) / D))
                ).astype(np.float32)
    t = np.arange(S, dtype=np.float32)
    freqs = np.outer(inv_freq, t).astype(np.float32)    # [64, S]
    cosT = np.concatenate([np.cos(freqs), np.cos(freqs)], 0).astype(ml_dtypes.bfloat16)
    sinS = np.concatenate([-np.sin(freqs), np.sin(freqs)], 0).astype(ml_dtypes.bfloat16)

    # shifted tril mask bank: tri[p, x] = 1 if p <= x - 384
    p = np.arange(128)[:, None]
    xx = np.arange(896)[None, :]
    tri = (p <= xx - 384).astype(ml_dtypes.bfloat16)

    esc = np.full((128, 1), s_p * s_p / np.sqrt(np.float32(D)), np.float32)
    osc = np.full((128, 1), s_p * s_o, np.float32)

    in_maps = []
    for c in range(NCORES):
        r = slice(c * HPC * D, (c + 1) * HPC * D)       # 256 features
        wt_c = np.ascontiguousarray(
            np.concatenate([tp[:H][r], tp[H:2 * H][r], tp[2 * H:][r]], 0).T
        ).astype(ml_dtypes.bfloat16)
        wot_c = np.ascontiguousarray(to[:, r].T).astype(ml_dtypes.bfloat16)
        in_maps.append({
            "xt": xt, "wt": wt_c, "wot": wot_c, "cost": cosT, "sins": sinS,
            "tri": tri, "osq": np.ones((128, 128), ml_dtypes.bfloat16),
            "esc": esc, "osc": osc,
        })
    return in_maps


def kernel(hidden_states, attention_mask, w_proj, w_o):
    global _built
    if _built is None:
        _built = _build()
    nc = _built
    in_maps = _host_prep(hidden_states, w_proj, w_o)
    res = run_bass_kernel_spmd(nc, in_maps, core_ids=list(range(NCORES)))
    acc = np.zeros((S, H), np.float32)
    for c in range(NCORES):
        acc += res.results[c]["out"].astype(np.float32)
    return acc.reshape(1, S, H)
